# revision 45
# baseline (speedup 1.0000x reference)
"""Multi-head attention (RoPE, causal) Trainium2 kernel.

Problem: B=2, L=2048, D=2048, H=16, dh=128, fp32.
Sharding: 8 cores = 2 batches x 4 head-groups (4 heads/core).
Each core computes QKV projections for its heads, RoPE, causal
attention, and a partial output projection (its heads' rows of Wo);
the host sums the 4 partials per batch.

Layout strategy (no on-device transposes of activations):
 - host uploads xT = x[b].T; Q/K produced transposed [d, l]; V natural
   [l, d]; scores computed transposed ST[k, q]; exp(ST) in [k, q] is
   directly the moving operand of the AV matmul with V as stationary,
   giving UT[d, q] - exactly the Wo-matmul stationary layout.
 - softmax without max subtraction (scores bounded, exp in bf16 whose
   range covers e^60). Row sums via matmuls with an ALL-ONES [128,128]
   stationary, accumulated in PSUM alongside AV: the sum lands
   replicated across all 128 PSUM partitions, so 1/r needs no
   partition broadcast - one DVE reciprocal + one DVE multiply
   normalizes UT during eviction. Full-width k-tiles are pre-summed
   in bf16 pairs on DVE so they need only one row-sum matmul per 2
   k-tiles; diagonal (causal-masked) tiles keep per-tile row sums.
 - causal diagonal blocks are trapezoids: ST/AV/row-sum matmuls
   restrict the moving operand to valid q >= 128*j (bf16 matmuls run
   1 cycle/row at any width), and exp+mask cover only that range.
 - RoPE rotate-half as a signed-permutation matmul (R^T stationary)
   after a host-side even/odd deinterleave of the Wq/Wk rows.

Dtypes: Q/K side (x, Wq, Wk, Wv, cos/sin, Q, K) in fp16; P/V side
(exp, V, at, Wo) in bf16 (exp needs bf16 range); PSUM f32. Measured
end-to-end error ~2.3e-3 vs the 2e-2 gate. fp16/bf16 matmuls run at
the same PE rate as f32r but halve DMA and SBUF, so ALL weights are
SBUF-resident, loaded once at startup across three DGE queues
(scalar/gpsimd/sync) staggered behind c0's x stream; after chunk 0
the kernel streams only x (2MB/chunk, host pre-tiled to [c][128,
KT*CHUNK] so each chunk is ONE full-bandwidth DMA prefetched during
the previous chunk's attention) and is never DMA-paced. Chunk 0's
~25us ramp is aggregate-HBM-bound (7MB of x+weights before qk can
finish) - per-kt tiles, chunk pairs, bulk rearranges, pre-tiled flat
layouts, and every queue assignment all measured within noise.

Perf structure (528us f32r baseline -> ~370us, best 369.0us):
 - attention processes heads in pairs with a one-iteration skew
   between the ST-matmul/exp stage and the AV/rowsum stage, so the
   tensor engine never stalls on the exp latency (stalls reset the PE
   p-state ramp: 1.2GHz for 3us after every gap, 2.4GHz after 3us of
   continuous execution).
 - PSUM: 3 "big" banks (proj accumulators / UT / out-proj) + 3 "st"
   banks (score tiles, rope rotate) + 2 "rb" banks (row sums) = 8.
 - out-proj eviction on DVE, stores alternate sync/scalar DGE queues;
   output partials stored bf16 (host sums in f32), halving store bytes.
   The first three out-proj tiles compute their pair-0 half as
   complete PSUM groups in the then-idle "st" banks (evicted to SBUF,
   summed with the pair-1 half on DVE) so the tensor engine has work
   while pair-1's normalization chain lands; "big"-bank fillers don't
   work (those slots are freed BY the chain) and "rb"-bank fillers
   measured worse.

Hardware pitfall found on the way: splitting one PSUM accumulation
group's matmuls into two rounds with other start=True matmuls to
OTHER banks interleaved between them corrupts results on HW (CoreSim
accepts it); keep each tile's accumulation contiguous per bank.
"""
import sys
import numpy as np

sys.path.insert(0, '/opt/trn_rl_repo')

import concourse.bass as bass  # noqa: E402,F401
import concourse.mybir as mybir  # noqa: E402
import concourse.tile as tile  # noqa: E402
from concourse import bacc  # noqa: E402
from concourse import library_config  # noqa: E402
from concourse.bass_utils import run_bass_kernel_spmd  # noqa: E402

B, L, D = 2, 2048, 2048
H, DH = 16, 128
HG = 4           # heads per core
G = H // HG      # head groups (cores per batch)
NCORES = 8
CHUNK = 512      # l-chunk
NCH = L // CHUNK          # 4 chunks
KT = D // 128             # 16 k-tiles over D
LT = L // 128             # 16 l-tiles
ROPE_BASE = 10000.0

f32 = mybir.dt.float32
f32r = mybir.dt.float32r
f16 = mybir.dt.float16
bf16 = mybir.dt.bfloat16

_built = None
PHASES = []


def _stage_weight_loads(nc, kt, wq_t, wq_d, wk_t, wk_d, wv_t, wv_d,
                        masks_t, mask_d, ones_c, ones_c_d):
    """One-time weight/constant loads staggered behind c0's x stream."""
    if kt == 3:
        nc.scalar.dma_start(out=wq_t[2][:], in_=wq_d[2])
        nc.scalar.dma_start(out=wq_t[3][:], in_=wq_d[3])
    elif kt == 7:
        nc.gpsimd.dma_start(out=wk_t[0][:], in_=wk_d[0])
        nc.gpsimd.dma_start(out=wk_t[1][:], in_=wk_d[1])
        nc.scalar.dma_start(out=wk_t[2][:], in_=wk_d[2])
        nc.scalar.dma_start(out=wk_t[3][:], in_=wk_d[3])
    elif kt == 11:
        nc.gpsimd.dma_start(out=wv_t[:, :8], in_=wv_d[:8].rearrange("k p f -> p k f"))
        nc.gpsimd.dma_start(out=wv_t[:, 8:], in_=wv_d[8:].rearrange("k p f -> p k f"))
    elif kt == 15:
        nc.sync.dma_start(out=masks_t[:],
                          in_=mask_d[:].rearrange("j p n -> p j n"))
        nc.sync.dma_start(out=ones_c[:], in_=ones_c_d[:])


def _build():
    nc = bacc.Bacc()

    # xt: [c][p][kt*CHUNK+n] = x[b].T[kt*128+p, c*CHUNK+n] (host pre-tiled
    # so every DMA line is >=4KB contiguous per partition)
    xt_d = nc.declare_dram_parameter("xt", [NCH, 128, KT * CHUNK], f16,
                                     isOutput=False)
    # wq/wk: [m][p][kt*128+f] = W^T[kt*128+p, m*128+f]
    wq_d = nc.declare_dram_parameter("wq", [HG, 128, KT * 128], f16, isOutput=False)
    wk_d = nc.declare_dram_parameter("wk", [HG, 128, KT * 128], f16, isOutput=False)
    # wv: [kt][p][f] = Wv^T[kt*128+p, f]
    wv_d = nc.declare_dram_parameter("wv", [KT, 128, HG * 128], f16, isOutput=False)
    wo_d = nc.declare_dram_parameter("wo", [HG, 128, D], bf16, isOutput=False)
    cos_d = nc.declare_dram_parameter("cosT", [128, L], f16, isOutput=False)
    sin_d = nc.declare_dram_parameter("sinT", [128, L], f16, isOutput=False)
    mask_d = nc.declare_dram_parameter("masks", [4, 128, CHUNK], bf16, isOutput=False)
    permr_d = nc.declare_dram_parameter("permr", [128, 128], f16, isOutput=False)
    ones_c_d = nc.declare_dram_parameter("ones_c", [128, 128], bf16, isOutput=False)

    out_d = nc.declare_dram_parameter("out", [L, D], bf16, isOutput=True)

    with tile.TileContext(nc) as tc:
        with (
            tc.tile_pool(name="const", bufs=1) as const,
            tc.tile_pool(name="persist", bufs=1) as persist,
            tc.tile_pool(name="xs", bufs=2) as xs,            # flat x tiles
            tc.tile_pool(name="chact", bufs=4) as chact,      # per-chunk qt/at
            tc.tile_pool(name="tmps", bufs=2) as tmps,        # transients
            tc.tile_pool(name="etp", bufs=6) as etp,          # exp tiles (bf16)
            tc.tile_pool(name="small", bufs=2) as small,      # [1,512] tiles
            tc.tile_pool(name="ps", bufs=1, space="PSUM") as pp,
        ):
            # ---- resident weights ----
            wq_t = [persist.tile([128, KT * 128], f16, name=f"wqt{m}")
                    for m in range(HG)]
            wk_t = [persist.tile([128, KT * 128], f16, name=f"wkt{m}")
                    for m in range(HG)]
            wv_t = persist.tile([128, KT, HG * 128], f16, name="wvt")
            wo_t = [persist.tile([128, D], bf16, name=f"wot{h}") for h in range(HG)]
            # all weight loads up front on the scalar DGE queue; they
            # stay ~2 strips ahead of the tensor engine through c0_qk
            nc.scalar.dma_start(out=wq_t[0][:], in_=wq_d[0])
            nc.scalar.dma_start(out=wq_t[1][:], in_=wq_d[1])

            # ---- constants (sync queue; permr needed by first rope) ----
            permr_t = const.tile([128, 128], f16)
            nc.sync.dma_start(out=permr_t[:], in_=permr_d[:])

            masks_t = const.tile([128, 4, CHUNK], bf16)
            ones_c = const.tile([128, 128], bf16)

            # ---- persistent activations (full history) ----
            kt_t = [persist.tile([128, L], f16, name=f"ktt{h}") for h in range(HG)]
            v_t = [persist.tile([128, HG * 128], bf16, name=f"vt{lt}")
                   for lt in range(LT)]

            xf_next = None
            for c in range(NCH):
                PHASES.append((f"c{c}_load", int(nc.next_id())))
                cs = slice(c * CHUNK, (c + 1) * CHUNK)
                # ---------- x for chunk c (flat pre-tiled, 4KB lines) ------
                if c == 0:
                    xf = xs.tile([128, KT * CHUNK], f16, tag="xf", name="xf0")
                    for q in range(4):
                        nc.sync.dma_start(
                            out=xf[:, q * 4 * CHUNK:(q + 1) * 4 * CHUNK],
                            in_=xt_d[0, :, q * 4 * CHUNK:(q + 1) * 4 * CHUNK])
                        _stage_weight_loads(nc, 4 * q + 3, wq_t, wq_d, wk_t,
                                            wk_d, wv_t, wv_d, masks_t, mask_d,
                                            ones_c, ones_c_d)
                else:
                    xf = xf_next
                xc = [xf[:, kt * CHUNK:(kt + 1) * CHUNK] for kt in range(KT)]
                cos_c = small.tile([128, CHUNK], f16, tag="cs", bufs=4)
                nc.scalar.dma_start(out=cos_c[:], in_=cos_d[:, cs])
                sin_c = small.tile([128, CHUNK], f16, tag="cs", bufs=4)
                nc.scalar.dma_start(out=sin_c[:], in_=sin_d[:, cs])

                PHASES.append((f"c{c}_qk", int(nc.next_id())))
                # ---------- Q/K projections + RoPE ----------
                qt_c = [chact.tile([128, CHUNK], f16, tag="qtc", name=f"qtc{h}")
                        for h in range(HG)]
                for (w_t_, isq) in ((wq_t, True), (wk_t, False)):
                    for m in range(HG):
                        wm = w_t_[m]
                        ps = pp.tile([128, CHUNK], f32, tag="big", bufs=3)
                        for kt in range(KT):
                            nc.tensor.matmul(ps[:], wm[:, kt * 128:(kt + 1) * 128],
                                             xc[kt][:],
                                             start=(kt == 0), stop=(kt == KT - 1))
                        # RoPE: out = raw*cos + (R @ raw)*sin
                        qraw = tmps.tile([128, CHUNK], f16, tag="qraw")
                        nc.scalar.copy(qraw[:], ps[:])
                        rot = pp.tile([128, CHUNK], f32, tag="st", bufs=3)
                        nc.tensor.matmul(rot[:], permr_t[:], qraw[:],
                                         start=True, stop=True)
                        t1 = tmps.tile([128, CHUNK], f16, tag="t1")
                        nc.vector.tensor_tensor(out=t1[:], in0=qraw[:],
                                                in1=cos_c[:],
                                                op=mybir.AluOpType.mult)
                        t2 = tmps.tile([128, CHUNK], f16, tag="t2")
                        nc.vector.tensor_tensor(out=t2[:], in0=rot[:],
                                                in1=sin_c[:],
                                                op=mybir.AluOpType.mult)
                        dst = qt_c[m] if isq else kt_t[m]
                        dst_ap = dst[:] if isq else dst[:, cs]
                        nc.vector.tensor_tensor(out=dst_ap, in0=t1[:], in1=t2[:],
                                                op=mybir.AluOpType.add)

                PHASES.append((f"c{c}_v", int(nc.next_id())))
                # ---------- V projection ----------
                for sl in range(CHUNK // 128):
                    lt = c * (CHUNK // 128) + sl
                    ps = pp.tile([128, HG * 128], f32, tag="big", bufs=3)
                    for kt in range(KT):
                        nc.tensor.matmul(
                            ps[:], xc[kt][:, sl * 128:(sl + 1) * 128],
                            wv_t[:, kt, :],
                            start=(kt == 0), stop=(kt == KT - 1))
                    nc.scalar.copy(v_t[lt][:], ps[:])

                PHASES.append((f"c{c}_attn", int(nc.next_id())))
                if c + 1 < NCH:
                    xf_next = xs.tile([128, KT * CHUNK], f16, tag="xf",
                                      name=f"xf{c + 1}")
                    nc.sync.dma_start(out=xf_next[:], in_=xt_d[c + 1])
                # ---------- attention for q-chunk c (head pairs, skewed) ----
                nkt = (c + 1) * (CHUNK // 128)   # causal: k-tiles 0..nkt-1
                at_c = [chact.tile([128, CHUNK], bf16, tag="atc", name=f"atc{h}")
                        for h in range(HG)]
                for pair in range(2):
                    hs = (2 * pair, 2 * pair + 1)
                    ut = {h: pp.tile([128, CHUNK], f32, tag="big", bufs=3,
                                     name=f"ut{h}") for h in hs}
                    rs = {h: pp.tile([128, CHUNK], f32, tag="rb", bufs=2,
                                     name=f"rs{h}") for h in hs}
                    ndiag = nkt - 4        # non-diag kts (multiple of 4)
                    ets = {}
                    for kt in range(nkt + 2):
                        if kt < nkt:
                            # double-wide exp tile: h0 in [:512], h1 in [512:]
                            et = etp.tile([128, 2 * CHUNK], bf16, tag="et")
                            diag_j = kt - ndiag
                            q0 = max(diag_j, 0) * 128   # trapezoid: valid q >= q0
                            for hi, h in enumerate(hs):
                                st = pp.tile([128, CHUNK], f32, tag="st", bufs=3)
                                nc.tensor.matmul(
                                    st[:, q0:], kt_t[h][:, kt * 128:(kt + 1) * 128],
                                    qt_c[h][:, q0:], start=True, stop=True)
                                esl = slice(hi * CHUNK + q0, (hi + 1) * CHUNK)
                                if diag_j >= 0:
                                    eraw = etp.tile([128, CHUNK], bf16, tag="eraw",
                                                    bufs=2)
                                    nc.scalar.activation(
                                        eraw[:, q0:], st[:, q0:],
                                        mybir.ActivationFunctionType.Exp)
                                    nc.vector.tensor_tensor(
                                        out=et[:, esl], in0=eraw[:, q0:],
                                        in1=masks_t[:, diag_j, q0:],
                                        op=mybir.AluOpType.mult)
                                else:
                                    nc.scalar.activation(
                                        et[:, esl], st[:, q0:],
                                        mybir.ActivationFunctionType.Exp)
                            ets[kt] = (et, q0)
                            # bf16 pair-sum of full-width kts on DVE; its
                            # row-sum needs only one matmul per 2 kts
                            if kt % 2 == 1 and kt < ndiag:
                                eps = etp.tile([128, 2 * CHUNK], bf16, tag="eps",
                                               bufs=3)
                                nc.vector.tensor_tensor(
                                    out=eps[:], in0=ets[kt - 1][0][:],
                                    in1=ets[kt][0][:], op=mybir.AluOpType.add)
                                ets[kt] = (ets[kt][0], 0, eps)
                        if kt >= 2:
                            j = kt - 2
                            entry = ets.pop(j)
                            e, eq0 = entry[0], entry[1]
                            first, last = j == 0, j == nkt - 1
                            for hi, h in enumerate(hs):
                                nc.tensor.matmul(
                                    ut[h][:, eq0:],
                                    v_t[j][:, h * 128:(h + 1) * 128],
                                    e[:, hi * CHUNK + eq0:(hi + 1) * CHUNK],
                                    start=first, stop=last)
                            if j < ndiag:
                                if j % 2 == 1:
                                    # odd tile: row sums via its pair-sum
                                    eps = entry[2]
                                    for hi, h in enumerate(hs):
                                        nc.tensor.matmul(
                                            rs[h][:], ones_c[:],
                                            eps[:, hi * CHUNK:(hi + 1) * CHUNK],
                                            start=(j == 1), stop=False)
                            else:
                                for hi, h in enumerate(hs):
                                    nc.tensor.matmul(
                                        rs[h][:, eq0:], ones_c[:],
                                        e[:, hi * CHUNK + eq0:(hi + 1) * CHUNK],
                                        start=first, stop=last)
                    for hi, h in enumerate(hs):
                        rb_sb = tmps.tile([128, CHUNK], f32, tag="bc", bufs=2)
                        nc.vector.reciprocal_approx_fast(out=rb_sb[:],
                                                         in_=rs[h][:])
                        nc.vector.tensor_tensor(out=at_c[h][:], in0=ut[h][:],
                                                in1=rb_sb[:],
                                                op=mybir.AluOpType.mult)

                PHASES.append((f"c{c}_out", int(nc.next_id())))
                # ---------- output projection for chunk c ----------
                if c == 0:
                    for h in range(HG):
                        nc.sync.dma_start(out=wo_t[h][:], in_=wo_d[h])
                # first two tiles: pair-0 halves as COMPLETE groups run
                # before pair-1's normalization chain lands (filling the
                # tensor gap); halves summed on DVE at eviction
                half_a = {}
                for idx in range(3):
                    ot, sl = divmod(idx, 4)
                    # "st" banks are free during out-proj, so these complete
                    # half-groups run before the pair-1 normalization chain
                    # frees any "big" bank
                    opa = pp.tile([128, 512], f32, tag="st", bufs=3,
                                  name=f"opa{idx}")
                    for h in (0, 1):
                        nc.tensor.matmul(
                            opa[:], at_c[h][:, sl * 128:(sl + 1) * 128],
                            wo_t[h][:, ot * 512:(ot + 1) * 512],
                            start=(h == 0), stop=(h == 1))
                    osa = tmps.tile([128, 512], f32, tag="osba", bufs=3)
                    nc.vector.tensor_copy(out=osa[:], in_=opa[:])
                    half_a[idx] = osa
                for idx in range(16):
                    ot, sl = divmod(idx, 4)
                    mt = c * (CHUNK // 128) + sl
                    osb = tmps.tile([128, 512], bf16, tag="osb", bufs=6)
                    if idx < 3:
                        opb = pp.tile([128, 512], f32, tag="big", bufs=3)
                        for h in (2, 3):
                            nc.tensor.matmul(
                                opb[:], at_c[h][:, sl * 128:(sl + 1) * 128],
                                wo_t[h][:, ot * 512:(ot + 1) * 512],
                                start=(h == 2), stop=(h == 3))
                        nc.vector.tensor_tensor(out=osb[:], in0=half_a[idx][:],
                                                in1=opb[:],
                                                op=mybir.AluOpType.add)
                    else:
                        ops = pp.tile([128, 512], f32, tag="big", bufs=3)
                        for h in range(HG):
                            nc.tensor.matmul(
                                ops[:], at_c[h][:, sl * 128:(sl + 1) * 128],
                                wo_t[h][:, ot * 512:(ot + 1) * 512],
                                start=(h == 0), stop=(h == HG - 1))
                        nc.vector.tensor_copy(out=osb[:], in_=ops[:])
                    qeng = (nc.sync, nc.scalar)[idx % 2]
                    qeng.dma_start(
                        out=out_d[mt * 128:(mt + 1) * 128, ot * 512:(ot + 1) * 512],
                        in_=osb[:])

    nc.finalize()
    return nc


def _get_nc():
    global _built
    if _built is None:
        _built = _build()
    return _built


def _host_prep(x, positions, Wq, Wk, Wv, Wo):
    """Build per-core input maps."""
    import ml_dtypes
    x = np.asarray(x, np.float32)
    positions = np.asarray(positions)
    Wq = np.asarray(Wq, np.float32)
    Wk = np.asarray(Wk, np.float32)
    Wv = np.asarray(Wv, np.float32)
    Wo = np.asarray(Wo, np.float32)

    scale = np.float32(1.0 / np.sqrt(DH))
    perm = np.concatenate([np.arange(0, DH, 2), np.arange(1, DH, 2)])  # deinterleave

    Wq_p = (Wq * scale).reshape(H, DH, D)[:, perm, :]   # [H, dh, D]
    Wk_p = Wk.reshape(H, DH, D)[:, perm, :]

    # RoPE tables per batch (deinterleaved: first 64 = even dims, last 64 = odd)
    inv_freq = 1.0 / (ROPE_BASE ** (np.arange(0, DH, 2, dtype=np.float32) / DH))
    cosT = np.empty((B, 128, L), np.float32)
    sinT = np.empty((B, 128, L), np.float32)
    for b in range(B):
        freqs = positions[b].astype(np.float32)[:, None] * inv_freq[None, :]  # [L, 64]
        cb = np.cos(freqs).T.astype(np.float32)  # [64, L]
        sb = np.sin(freqs).T.astype(np.float32)
        cosT[b] = np.concatenate([cb, cb], axis=0)
        sinT[b] = np.concatenate([sb, sb], axis=0)

    # rotate-half signed permutation (in deinterleaved space), uploaded as R.T
    R = np.zeros((128, 128), np.float32)
    for i in range(64):
        R[i, i + 64] = -1.0
        R[i + 64, i] = 1.0
    permr = R.T.astype(np.float16)

    # causal masks for diagonal blocks (0/1, exact in bf16)
    masks = np.zeros((4, 128, CHUNK), np.float32)
    for j in range(4):
        kk = j * 128 + np.arange(128)[:, None]
        qq = np.arange(CHUNK)[None, :]
        masks[j] = (kk <= qq).astype(np.float32)
    masks = masks.astype(ml_dtypes.bfloat16)

    ones_c = np.ones((128, 128), ml_dtypes.bfloat16)

    in_maps = []
    for core in range(NCORES):
        b, g = divmod(core, G)
        hs = slice(g * HG, (g + 1) * HG)
        # W^T for this core's heads: [D, HG*dh]
        wqT = Wq_p[hs].reshape(HG * DH, D).T          # [D, 512]
        wkT = Wk_p[hs].reshape(HG * DH, D).T
        wvT = Wv.reshape(H, DH, D)[hs].reshape(HG * DH, D).T
        # [m][p][kt*128+f] layout
        wq_c = np.ascontiguousarray(
            wqT.reshape(KT, 128, HG, DH).transpose(2, 1, 0, 3).reshape(
                HG, 128, KT * DH)).astype(np.float16)
        wk_c = np.ascontiguousarray(
            wkT.reshape(KT, 128, HG, DH).transpose(2, 1, 0, 3).reshape(
                HG, 128, KT * DH)).astype(np.float16)
        # [kt][p][f]
        wv_c = np.ascontiguousarray(
            wvT.reshape(KT, 128, HG * DH)).astype(np.float16)
        # wo[h][d'][o] = Wo[o, (g*HG+h)*dh + d']
        wo_c = np.ascontiguousarray(
            Wo.T.reshape(H, DH, D)[hs]).astype(ml_dtypes.bfloat16)  # [HG, dh, D]
        xtb = x[b].T.astype(np.float16)   # [D, L]
        xt_tiled = np.ascontiguousarray(
            xtb.reshape(KT, 128, NCH, CHUNK).transpose(2, 1, 0, 3).reshape(
                NCH, 128, KT * CHUNK))
        in_maps.append({
            "xt": xt_tiled,
            "wq": wq_c, "wk": wk_c, "wv": wv_c, "wo": wo_c,
            "cosT": cosT[b].astype(np.float16),
            "sinT": sinT[b].astype(np.float16),
            "masks": masks, "permr": permr,
            "ones_c": ones_c,
        })
    return in_maps


def kernel(x, positions, Wq, Wk, Wv, Wo, _profile=False):
    nc = _get_nc()
    in_maps = _host_prep(x, positions, Wq, Wk, Wv, Wo)
    res = run_bass_kernel_spmd(nc, in_maps, list(range(NCORES)), trace=_profile)
    out = np.zeros((B, L, D), np.float32)
    for core in range(NCORES):
        b = core // G
        out[b] += np.asarray(res.results[core]["out"], np.float32)
    if _profile:
        kernel._last_exec_time_ns = res.exec_time_ns
        kernel._last_trace = res.instructions_and_trace
    return out


# revision 46
# speedup vs baseline: 1.0025x; 1.0025x over previous
"""Multi-head attention (RoPE, causal) Trainium2 kernel.

Problem: B=2, L=2048, D=2048, H=16, dh=128, fp32.
Sharding: 8 cores = 2 batches x 4 head-groups (4 heads/core).
Each core computes QKV projections for its heads, RoPE, causal
attention, and a partial output projection (its heads' rows of Wo);
the host sums the 4 partials per batch.

Layout strategy (no on-device transposes of activations):
 - host uploads xT = x[b].T; Q/K produced transposed [d, l]; V natural
   [l, d]; scores computed transposed ST[k, q]; exp(ST) in [k, q] is
   directly the moving operand of the AV matmul with V as stationary,
   giving UT[d, q] - exactly the Wo-matmul stationary layout.
 - softmax without max subtraction (scores bounded, exp in bf16 whose
   range covers e^60). Row sums via matmuls with an ALL-ONES [128,128]
   stationary, accumulated in PSUM alongside AV: the sum lands
   replicated across all 128 PSUM partitions, so 1/r needs no
   partition broadcast - one DVE reciprocal + one DVE multiply
   normalizes UT during eviction. Full-width k-tiles are pre-summed
   in bf16 pairs on DVE so they need only one row-sum matmul per 2
   k-tiles; diagonal (causal-masked) tiles keep per-tile row sums.
 - causal diagonal blocks are trapezoids: ST/AV/row-sum matmuls
   restrict the moving operand to valid q >= 128*j (bf16 matmuls run
   1 cycle/row at any width), and exp+mask cover only that range.
 - RoPE rotate-half as a signed-permutation matmul (R^T stationary)
   after a host-side even/odd deinterleave of the Wq/Wk rows.

Dtypes: Q/K side (x, Wq, Wk, Wv, cos/sin, Q, K) in fp16; P/V side
(exp, V, at, Wo) in bf16 (exp needs bf16 range); PSUM f32. Measured
end-to-end error ~2.3e-3 vs the 2e-2 gate. fp16/bf16 matmuls run at
the same PE rate as f32r but halve DMA and SBUF, so ALL weights are
SBUF-resident, loaded once at startup across three DGE queues
(scalar/gpsimd/sync) staggered behind c0's x stream; after chunk 0
the kernel streams only x (2MB/chunk, host pre-tiled to [c][128,
KT*CHUNK] so each chunk is ONE full-bandwidth DMA prefetched during
the previous chunk's attention) and is never DMA-paced. Chunk 0's
~25us ramp is aggregate-HBM-bound (7MB of x+weights before qk can
finish) - per-kt tiles, chunk pairs, bulk rearranges, pre-tiled flat
layouts, and every queue assignment all measured within noise.

Perf structure (528us f32r baseline -> ~370us, best 369.0us):
 - attention processes heads in pairs with a one-iteration skew
   between the ST-matmul/exp stage and the AV/rowsum stage, so the
   tensor engine never stalls on the exp latency (stalls reset the PE
   p-state ramp: 1.2GHz for 3us after every gap, 2.4GHz after 3us of
   continuous execution).
 - PSUM: 3 "big" banks (proj accumulators / UT / out-proj) + 3 "st"
   banks (score tiles, rope rotate) + 2 "rb" banks (row sums) = 8.
 - out-proj eviction on DVE, stores alternate sync/scalar DGE queues;
   output partials stored bf16 (host sums in f32), halving store bytes.
   The first three out-proj tiles compute their pair-0 half as
   complete PSUM groups in the then-idle "st" banks (evicted to SBUF,
   summed with the pair-1 half on DVE) so the tensor engine has work
   while pair-1's normalization chain lands; "big"-bank fillers don't
   work (those slots are freed BY the chain) and "rb"-bank fillers
   measured worse.

Hardware pitfall found on the way: splitting one PSUM accumulation
group's matmuls into two rounds with other start=True matmuls to
OTHER banks interleaved between them corrupts results on HW (CoreSim
accepts it); keep each tile's accumulation contiguous per bank.
"""
import sys
import numpy as np

sys.path.insert(0, '/opt/trn_rl_repo')

import concourse.bass as bass  # noqa: E402,F401
import concourse.mybir as mybir  # noqa: E402
import concourse.tile as tile  # noqa: E402
from concourse import bacc  # noqa: E402
from concourse import library_config  # noqa: E402
from concourse.bass_utils import run_bass_kernel_spmd  # noqa: E402

B, L, D = 2, 2048, 2048
H, DH = 16, 128
HG = 4           # heads per core
G = H // HG      # head groups (cores per batch)
NCORES = 8
CHUNK = 512      # l-chunk
NCH = L // CHUNK          # 4 chunks
KT = D // 128             # 16 k-tiles over D
LT = L // 128             # 16 l-tiles
ROPE_BASE = 10000.0

f32 = mybir.dt.float32
f32r = mybir.dt.float32r
f16 = mybir.dt.float16
bf16 = mybir.dt.bfloat16

_built = None
PHASES = []


def _stage_weight_loads(nc, kt, wq_t, wq_d, wk_t, wk_d, wv_t, wv_d,
                        masks_t, mask_d, ones_c, ones_c_d):
    """One-time weight/constant loads staggered behind c0's x stream."""
    if kt == 3:
        nc.scalar.dma_start(out=wq_t[2][:], in_=wq_d[2])
        nc.scalar.dma_start(out=wq_t[3][:], in_=wq_d[3])
    elif kt == 7:
        nc.gpsimd.dma_start(out=wk_t[0][:], in_=wk_d[0])
        nc.gpsimd.dma_start(out=wk_t[1][:], in_=wk_d[1])
        nc.scalar.dma_start(out=wk_t[2][:], in_=wk_d[2])
        nc.scalar.dma_start(out=wk_t[3][:], in_=wk_d[3])
    elif kt == 11:
        nc.gpsimd.dma_start(out=wv_t[:, :8], in_=wv_d[:8].rearrange("k p f -> p k f"))
        nc.gpsimd.dma_start(out=wv_t[:, 8:], in_=wv_d[8:].rearrange("k p f -> p k f"))
    elif kt == 15:
        nc.sync.dma_start(out=masks_t[:],
                          in_=mask_d[:].rearrange("j p n -> p j n"))
        nc.sync.dma_start(out=ones_c[:], in_=ones_c_d[:])


def _build():
    nc = bacc.Bacc()

    # xt: [c][p][kt*CHUNK+n] = x[b].T[kt*128+p, c*CHUNK+n] (host pre-tiled
    # so every DMA line is >=4KB contiguous per partition)
    xt_d = nc.declare_dram_parameter("xt", [NCH, 128, KT * CHUNK], f16,
                                     isOutput=False)
    # wq/wk: [m][p][kt*128+f] = W^T[kt*128+p, m*128+f]
    wq_d = nc.declare_dram_parameter("wq", [HG, 128, KT * 128], f16, isOutput=False)
    wk_d = nc.declare_dram_parameter("wk", [HG, 128, KT * 128], f16, isOutput=False)
    # wv: [kt][p][f] = Wv^T[kt*128+p, f]
    wv_d = nc.declare_dram_parameter("wv", [KT, 128, HG * 128], f16, isOutput=False)
    wo_d = nc.declare_dram_parameter("wo", [HG, 128, D], bf16, isOutput=False)
    cos_d = nc.declare_dram_parameter("cosT", [128, L], f16, isOutput=False)
    sin_d = nc.declare_dram_parameter("sinT", [128, L], f16, isOutput=False)
    mask_d = nc.declare_dram_parameter("masks", [4, 128, CHUNK], bf16, isOutput=False)
    permr_d = nc.declare_dram_parameter("permr", [128, 128], f16, isOutput=False)
    ones_c_d = nc.declare_dram_parameter("ones_c", [128, 128], bf16, isOutput=False)

    out_d = nc.declare_dram_parameter("out", [L, D], bf16, isOutput=True)

    with tile.TileContext(nc) as tc:
        with (
            tc.tile_pool(name="const", bufs=1) as const,
            tc.tile_pool(name="persist", bufs=1) as persist,
            tc.tile_pool(name="xs", bufs=2) as xs,            # flat x tiles
            tc.tile_pool(name="chact", bufs=4) as chact,      # per-chunk qt/at
            tc.tile_pool(name="tmps", bufs=2) as tmps,        # transients
            tc.tile_pool(name="etp", bufs=6) as etp,          # exp tiles (bf16)
            tc.tile_pool(name="small", bufs=2) as small,      # [1,512] tiles
            tc.tile_pool(name="ps", bufs=1, space="PSUM") as pp,
        ):
            # ---- resident weights ----
            wq_t = [persist.tile([128, KT * 128], f16, name=f"wqt{m}")
                    for m in range(HG)]
            wk_t = [persist.tile([128, KT * 128], f16, name=f"wkt{m}")
                    for m in range(HG)]
            wv_t = persist.tile([128, KT, HG * 128], f16, name="wvt")
            wo_t = [persist.tile([128, D], bf16, name=f"wot{h}") for h in range(HG)]
            # all weight loads up front on the scalar DGE queue; they
            # stay ~2 strips ahead of the tensor engine through c0_qk
            nc.scalar.dma_start(out=wq_t[0][:], in_=wq_d[0])
            nc.scalar.dma_start(out=wq_t[1][:], in_=wq_d[1])

            # ---- constants (sync queue; permr needed by first rope) ----
            permr_t = const.tile([128, 128], f16)
            nc.sync.dma_start(out=permr_t[:], in_=permr_d[:])

            masks_t = const.tile([128, 4, CHUNK], bf16)
            ones_c = const.tile([128, 128], bf16)

            # ---- persistent activations (full history) ----
            kt_t = [persist.tile([128, L], f16, name=f"ktt{h}") for h in range(HG)]
            v_t = [persist.tile([128, HG * 128], bf16, name=f"vt{lt}")
                   for lt in range(LT)]

            xf_next = None
            for c in range(NCH):
                PHASES.append((f"c{c}_load", int(nc.next_id())))
                cs = slice(c * CHUNK, (c + 1) * CHUNK)
                # ---------- x for chunk c (flat pre-tiled, 4KB lines) ------
                if c == 0:
                    xf = xs.tile([128, KT * CHUNK], f16, tag="xf", name="xf0")
                    for q in range(4):
                        nc.sync.dma_start(
                            out=xf[:, q * 4 * CHUNK:(q + 1) * 4 * CHUNK],
                            in_=xt_d[0, :, q * 4 * CHUNK:(q + 1) * 4 * CHUNK])
                        _stage_weight_loads(nc, 4 * q + 3, wq_t, wq_d, wk_t,
                                            wk_d, wv_t, wv_d, masks_t, mask_d,
                                            ones_c, ones_c_d)
                else:
                    xf = xf_next
                xc = [xf[:, kt * CHUNK:(kt + 1) * CHUNK] for kt in range(KT)]
                cos_c = small.tile([128, CHUNK], f16, tag="cs", bufs=4)
                nc.scalar.dma_start(out=cos_c[:], in_=cos_d[:, cs])
                sin_c = small.tile([128, CHUNK], f16, tag="cs", bufs=4)
                nc.scalar.dma_start(out=sin_c[:], in_=sin_d[:, cs])

                PHASES.append((f"c{c}_qk", int(nc.next_id())))
                # ---------- Q/K projections + RoPE ----------
                qt_c = [chact.tile([128, CHUNK], f16, tag="qtc", name=f"qtc{h}")
                        for h in range(HG)]
                for (w_t_, isq) in ((wq_t, True), (wk_t, False)):
                    for m in range(HG):
                        wm = w_t_[m]
                        ps = pp.tile([128, CHUNK], f32, tag="big", bufs=3)
                        for kt in range(KT):
                            nc.tensor.matmul(ps[:], wm[:, kt * 128:(kt + 1) * 128],
                                             xc[kt][:],
                                             start=(kt == 0), stop=(kt == KT - 1))
                        # RoPE: out = raw*cos + (R @ raw)*sin
                        qraw = tmps.tile([128, CHUNK], f16, tag="qraw")
                        nc.scalar.copy(qraw[:], ps[:])
                        rot = pp.tile([128, CHUNK], f32, tag="st", bufs=3)
                        nc.tensor.matmul(rot[:], permr_t[:], qraw[:],
                                         start=True, stop=True)
                        t1 = tmps.tile([128, CHUNK], f16, tag="t1")
                        nc.vector.tensor_tensor(out=t1[:], in0=qraw[:],
                                                in1=cos_c[:],
                                                op=mybir.AluOpType.mult)
                        t2 = tmps.tile([128, CHUNK], f16, tag="t2")
                        nc.vector.tensor_tensor(out=t2[:], in0=rot[:],
                                                in1=sin_c[:],
                                                op=mybir.AluOpType.mult)
                        dst = qt_c[m] if isq else kt_t[m]
                        dst_ap = dst[:] if isq else dst[:, cs]
                        nc.vector.tensor_tensor(out=dst_ap, in0=t1[:], in1=t2[:],
                                                op=mybir.AluOpType.add)

                PHASES.append((f"c{c}_v", int(nc.next_id())))
                # ---------- V projection ----------
                for sl in range(CHUNK // 128):
                    lt = c * (CHUNK // 128) + sl
                    ps = pp.tile([128, HG * 128], f32, tag="big", bufs=3)
                    for kt in range(KT):
                        nc.tensor.matmul(
                            ps[:], xc[kt][:, sl * 128:(sl + 1) * 128],
                            wv_t[:, kt, :],
                            start=(kt == 0), stop=(kt == KT - 1))
                    nc.scalar.copy(v_t[lt][:], ps[:])

                PHASES.append((f"c{c}_attn", int(nc.next_id())))
                if c + 1 < NCH:
                    xf_next = xs.tile([128, KT * CHUNK], f16, tag="xf",
                                      name=f"xf{c + 1}")
                    nc.sync.dma_start(out=xf_next[:], in_=xt_d[c + 1])
                # ---------- attention for q-chunk c (head pairs, skewed) ----
                nkt = (c + 1) * (CHUNK // 128)   # causal: k-tiles 0..nkt-1
                at_c = [chact.tile([128, CHUNK], bf16, tag="atc", name=f"atc{h}")
                        for h in range(HG)]
                for pair in range(2):
                    hs = (2 * pair, 2 * pair + 1)
                    ut = {h: pp.tile([128, CHUNK], f32, tag="big", bufs=3,
                                     name=f"ut{h}") for h in hs}
                    rs = {h: pp.tile([128, CHUNK], f32, tag="rb", bufs=2,
                                     name=f"rs{h}") for h in hs}
                    ndiag = nkt - 4        # non-diag kts (multiple of 4)
                    ets = {}
                    for kt in range(nkt + 1):
                        if kt < nkt:
                            # double-wide exp tile: h0 in [:512], h1 in [512:]
                            et = etp.tile([128, 2 * CHUNK], bf16, tag="et")
                            diag_j = kt - ndiag
                            q0 = max(diag_j, 0) * 128   # trapezoid: valid q >= q0
                            for hi, h in enumerate(hs):
                                st = pp.tile([128, CHUNK], f32, tag="st", bufs=3)
                                nc.tensor.matmul(
                                    st[:, q0:], kt_t[h][:, kt * 128:(kt + 1) * 128],
                                    qt_c[h][:, q0:], start=True, stop=True)
                                esl = slice(hi * CHUNK + q0, (hi + 1) * CHUNK)
                                if diag_j >= 0:
                                    eraw = etp.tile([128, CHUNK], bf16, tag="eraw",
                                                    bufs=2)
                                    nc.scalar.activation(
                                        eraw[:, q0:], st[:, q0:],
                                        mybir.ActivationFunctionType.Exp)
                                    nc.vector.tensor_tensor(
                                        out=et[:, esl], in0=eraw[:, q0:],
                                        in1=masks_t[:, diag_j, q0:],
                                        op=mybir.AluOpType.mult)
                                else:
                                    nc.scalar.activation(
                                        et[:, esl], st[:, q0:],
                                        mybir.ActivationFunctionType.Exp)
                            ets[kt] = (et, q0)
                            # bf16 pair-sum of full-width kts on DVE; its
                            # row-sum needs only one matmul per 2 kts
                            if kt % 2 == 1 and kt < ndiag:
                                eps = etp.tile([128, 2 * CHUNK], bf16, tag="eps",
                                               bufs=3)
                                nc.vector.tensor_tensor(
                                    out=eps[:], in0=ets[kt - 1][0][:],
                                    in1=ets[kt][0][:], op=mybir.AluOpType.add)
                                ets[kt] = (ets[kt][0], 0, eps)
                        if kt >= 1:
                            j = kt - 1
                            entry = ets.pop(j)
                            e, eq0 = entry[0], entry[1]
                            first, last = j == 0, j == nkt - 1
                            for hi, h in enumerate(hs):
                                nc.tensor.matmul(
                                    ut[h][:, eq0:],
                                    v_t[j][:, h * 128:(h + 1) * 128],
                                    e[:, hi * CHUNK + eq0:(hi + 1) * CHUNK],
                                    start=first, stop=last)
                            if j < ndiag:
                                if j % 2 == 1:
                                    # odd tile: row sums via its pair-sum
                                    eps = entry[2]
                                    for hi, h in enumerate(hs):
                                        nc.tensor.matmul(
                                            rs[h][:], ones_c[:],
                                            eps[:, hi * CHUNK:(hi + 1) * CHUNK],
                                            start=(j == 1), stop=False)
                            else:
                                for hi, h in enumerate(hs):
                                    nc.tensor.matmul(
                                        rs[h][:, eq0:], ones_c[:],
                                        e[:, hi * CHUNK + eq0:(hi + 1) * CHUNK],
                                        start=first, stop=last)
                    for hi, h in enumerate(hs):
                        rb_sb = tmps.tile([128, CHUNK], f32, tag="bc", bufs=2)
                        nc.vector.reciprocal_approx_fast(out=rb_sb[:],
                                                         in_=rs[h][:])
                        nc.vector.tensor_tensor(out=at_c[h][:], in0=ut[h][:],
                                                in1=rb_sb[:],
                                                op=mybir.AluOpType.mult)

                PHASES.append((f"c{c}_out", int(nc.next_id())))
                # ---------- output projection for chunk c ----------
                if c == 0:
                    for h in range(HG):
                        nc.sync.dma_start(out=wo_t[h][:], in_=wo_d[h])
                # first two tiles: pair-0 halves as COMPLETE groups run
                # before pair-1's normalization chain lands (filling the
                # tensor gap); halves summed on DVE at eviction
                half_a = {}
                for idx in range(3):
                    ot, sl = divmod(idx, 4)
                    # "st" banks are free during out-proj, so these complete
                    # half-groups run before the pair-1 normalization chain
                    # frees any "big" bank
                    opa = pp.tile([128, 512], f32, tag="st", bufs=3,
                                  name=f"opa{idx}")
                    for h in (0, 1):
                        nc.tensor.matmul(
                            opa[:], at_c[h][:, sl * 128:(sl + 1) * 128],
                            wo_t[h][:, ot * 512:(ot + 1) * 512],
                            start=(h == 0), stop=(h == 1))
                    osa = tmps.tile([128, 512], f32, tag="osba", bufs=3)
                    nc.vector.tensor_copy(out=osa[:], in_=opa[:])
                    half_a[idx] = osa
                for idx in range(16):
                    ot, sl = divmod(idx, 4)
                    mt = c * (CHUNK // 128) + sl
                    osb = tmps.tile([128, 512], bf16, tag="osb", bufs=6)
                    if idx < 3:
                        opb = pp.tile([128, 512], f32, tag="big", bufs=3)
                        for h in (2, 3):
                            nc.tensor.matmul(
                                opb[:], at_c[h][:, sl * 128:(sl + 1) * 128],
                                wo_t[h][:, ot * 512:(ot + 1) * 512],
                                start=(h == 2), stop=(h == 3))
                        nc.vector.tensor_tensor(out=osb[:], in0=half_a[idx][:],
                                                in1=opb[:],
                                                op=mybir.AluOpType.add)
                    else:
                        ops = pp.tile([128, 512], f32, tag="big", bufs=3)
                        for h in range(HG):
                            nc.tensor.matmul(
                                ops[:], at_c[h][:, sl * 128:(sl + 1) * 128],
                                wo_t[h][:, ot * 512:(ot + 1) * 512],
                                start=(h == 0), stop=(h == HG - 1))
                        nc.vector.tensor_copy(out=osb[:], in_=ops[:])
                    qeng = (nc.sync, nc.scalar)[idx % 2]
                    qeng.dma_start(
                        out=out_d[mt * 128:(mt + 1) * 128, ot * 512:(ot + 1) * 512],
                        in_=osb[:])

    nc.finalize()
    return nc


def _get_nc():
    global _built
    if _built is None:
        _built = _build()
    return _built


def _host_prep(x, positions, Wq, Wk, Wv, Wo):
    """Build per-core input maps."""
    import ml_dtypes
    x = np.asarray(x, np.float32)
    positions = np.asarray(positions)
    Wq = np.asarray(Wq, np.float32)
    Wk = np.asarray(Wk, np.float32)
    Wv = np.asarray(Wv, np.float32)
    Wo = np.asarray(Wo, np.float32)

    scale = np.float32(1.0 / np.sqrt(DH))
    perm = np.concatenate([np.arange(0, DH, 2), np.arange(1, DH, 2)])  # deinterleave

    Wq_p = (Wq * scale).reshape(H, DH, D)[:, perm, :]   # [H, dh, D]
    Wk_p = Wk.reshape(H, DH, D)[:, perm, :]

    # RoPE tables per batch (deinterleaved: first 64 = even dims, last 64 = odd)
    inv_freq = 1.0 / (ROPE_BASE ** (np.arange(0, DH, 2, dtype=np.float32) / DH))
    cosT = np.empty((B, 128, L), np.float32)
    sinT = np.empty((B, 128, L), np.float32)
    for b in range(B):
        freqs = positions[b].astype(np.float32)[:, None] * inv_freq[None, :]  # [L, 64]
        cb = np.cos(freqs).T.astype(np.float32)  # [64, L]
        sb = np.sin(freqs).T.astype(np.float32)
        cosT[b] = np.concatenate([cb, cb], axis=0)
        sinT[b] = np.concatenate([sb, sb], axis=0)

    # rotate-half signed permutation (in deinterleaved space), uploaded as R.T
    R = np.zeros((128, 128), np.float32)
    for i in range(64):
        R[i, i + 64] = -1.0
        R[i + 64, i] = 1.0
    permr = R.T.astype(np.float16)

    # causal masks for diagonal blocks (0/1, exact in bf16)
    masks = np.zeros((4, 128, CHUNK), np.float32)
    for j in range(4):
        kk = j * 128 + np.arange(128)[:, None]
        qq = np.arange(CHUNK)[None, :]
        masks[j] = (kk <= qq).astype(np.float32)
    masks = masks.astype(ml_dtypes.bfloat16)

    ones_c = np.ones((128, 128), ml_dtypes.bfloat16)

    in_maps = []
    for core in range(NCORES):
        b, g = divmod(core, G)
        hs = slice(g * HG, (g + 1) * HG)
        # W^T for this core's heads: [D, HG*dh]
        wqT = Wq_p[hs].reshape(HG * DH, D).T          # [D, 512]
        wkT = Wk_p[hs].reshape(HG * DH, D).T
        wvT = Wv.reshape(H, DH, D)[hs].reshape(HG * DH, D).T
        # [m][p][kt*128+f] layout
        wq_c = np.ascontiguousarray(
            wqT.reshape(KT, 128, HG, DH).transpose(2, 1, 0, 3).reshape(
                HG, 128, KT * DH)).astype(np.float16)
        wk_c = np.ascontiguousarray(
            wkT.reshape(KT, 128, HG, DH).transpose(2, 1, 0, 3).reshape(
                HG, 128, KT * DH)).astype(np.float16)
        # [kt][p][f]
        wv_c = np.ascontiguousarray(
            wvT.reshape(KT, 128, HG * DH)).astype(np.float16)
        # wo[h][d'][o] = Wo[o, (g*HG+h)*dh + d']
        wo_c = np.ascontiguousarray(
            Wo.T.reshape(H, DH, D)[hs]).astype(ml_dtypes.bfloat16)  # [HG, dh, D]
        xtb = x[b].T.astype(np.float16)   # [D, L]
        xt_tiled = np.ascontiguousarray(
            xtb.reshape(KT, 128, NCH, CHUNK).transpose(2, 1, 0, 3).reshape(
                NCH, 128, KT * CHUNK))
        in_maps.append({
            "xt": xt_tiled,
            "wq": wq_c, "wk": wk_c, "wv": wv_c, "wo": wo_c,
            "cosT": cosT[b].astype(np.float16),
            "sinT": sinT[b].astype(np.float16),
            "masks": masks, "permr": permr,
            "ones_c": ones_c,
        })
    return in_maps


def kernel(x, positions, Wq, Wk, Wv, Wo, _profile=False):
    nc = _get_nc()
    in_maps = _host_prep(x, positions, Wq, Wk, Wv, Wo)
    res = run_bass_kernel_spmd(nc, in_maps, list(range(NCORES)), trace=_profile)
    out = np.zeros((B, L, D), np.float32)
    for core in range(NCORES):
        b = core // G
        out[b] += np.asarray(res.results[core]["out"], np.float32)
    if _profile:
        kernel._last_exec_time_ns = res.exec_time_ns
        kernel._last_trace = res.instructions_and_trace
    return out
